# revision 7
# baseline (speedup 1.0000x reference)
"""Trainium2 Bass kernel for the 3-view attention-fusion pooling module.

Computation (reference):
    t_k  = tanh(W @ x_k)                      (A=256, D=256), k = 1..3
    s_k  = h_n @ t_k                          (1, D)
    beta = softmax([s_1; s_2; s_3], axis=0)   (3, D)
    out  = beta[0]*x1 + beta[1]*x2 + beta[2]*x3   (N, D)

Strategy (fp16, SBUF-resident x):
  * Rows (node dim N=100000) split across 8 cores; W fed per-core as
    W[:, shard].T so the contraction dim is the partition dim.
  * All inputs are converted to fp16 on the host: halves HBM read traffic
    (the kernel is memory-regime) and doubles PE throughput. Host-side
    numpy validation puts the end-to-end rel err at ~6e-4 (gate 2e-2).
  * x1/x2/x3 are held fully resident in SBUF (3 x 50 KiB/partition), so
    phase 2 (the beta-weighted sum) re-reads nothing from HBM.
  * While phase 1 streams, the idle DVE precomputes d1 = x1-x3,
    d2 = x2-x3 in place; with sum(beta)=1 phase 2 is then
    out = x3 + beta0*d1 + beta1*d2 (4 elementwise ops, not 5).
  * The (A, 3D) GEMM partials are AllReduce-summed in fp16 across the 8
    cores. Measured on this runtime the collective has a ~160 us FIXED
    cost (payload nearly irrelevant: 4 KB vs 393 KB differ by ~38 us),
    so exactly ONE collective is issued; splitting or pipelining it only
    multiplies the fixed cost. Everything downstream is tiny.
  * Phase-2 elementwise is split DVE/Pool ~3.5:1 (Pool TT is ~3.5x
    slower); beta is materialized as flat contiguous fp16 operands so
    the DVE keeps its 2x packed mode.
  * out is written as fp16 and upcast to fp32 on the host (halves the
    HBM write traffic).

Layout: within a batch of P*R rows, partition p holds R consecutive DRAM
rows, so every DMA moves R*D*2 contiguous bytes per partition (20 KiB -
1.28 MiB per dma_start, near line-rate).
"""

import sys

import numpy as np

for _p in ("/opt/trn_rl_repo", "/root/.axon_site/_ro/trn_rl_repo"):
    if _p not in sys.path:
        sys.path.append(_p)

import concourse.bacc as bacc
import concourse.tile as tile
from concourse import mybir
from concourse.bass_utils import run_bass_kernel_spmd

N_CORES = 8
N = 100000
D = 256          # feature dim
A = 256          # input_att
N_LOC = N // N_CORES   # 12500 rows per core
P = 125          # partitions per batch (matmul contraction chunk)
R = 20           # rows per partition per batch
NB = N_LOC // (P * R)  # 5 batches
FW = R * D       # free width of a batched SBUF tile (fp16 elements)
HW = FW // 2     # phase-2 half-batch width

FP32 = mybir.dt.float32
FP16 = mybir.dt.float16


def _emit_iteration(nc, tc, rep, xrs, wtr, outr, hn16, ones_sb, pdram,
                    n_cores, collective, phase2=True, dtrick=True,
                    phase1_only=False, ar_cols=None, no_pool=False,
                    p2_full=True, p2_tiny_out=False):
    Tanh = mybir.ActivationFunctionType.Tanh
    Exp = mybir.ActivationFunctionType.Exp
    r = rep

    with (
        tc.tile_pool(name=f"px1_{r}", bufs=NB) as px1,
        tc.tile_pool(name=f"px2_{r}", bufs=NB) as px2,
        tc.tile_pool(name=f"px3_{r}", bufs=NB) as px3,
        tc.tile_pool(name=f"small_{r}", bufs=1) as small,
    ):
        xpools = (px1, px2, px3)
        xt = {}

        # ---------------- phase 1: u_k = W @ x_k (per-core partials) -------
        # fp16 collective payload: the 8-way sum of fp16-rounded partials
        # costs ~0.05% relative error, well inside budget
        cc_in = small.tile([128, 6 * D], FP16, name="cc_in", tag="cc_in")
        with (
            tc.tile_pool(name=f"pacc_{r}", bufs=1, space="PSUM") as pacc,
            tc.tile_pool(name=f"pw_{r}", bufs=2) as pw,
        ):
            uacc = [[pacc.tile([128, D], FP32, name=f"u{v}{h}",
                               tag=f"u{v}{h}")
                     for h in range(2)] for v in range(3)]
            for b in range(NB):
                xts = []
                for v in range(3):
                    t = xpools[v].tile([P, FW], FP16, name=f"x{v}", tag="x")
                    # split loads across both HWDGE rings (SP + ACT)
                    eng = nc.sync if v < 2 else nc.scalar
                    eng.dma_start(t[:], xrs[v][b])
                    xts.append(t)
                    xt[(v, b)] = t
                wtile = pw.tile([P, FW], FP16, name="w", tag="w")
                nc.scalar.dma_start(wtile[:], wtr[b])
                for g in range(R):
                    first = (b == 0 and g == 0)
                    last = (b == NB - 1 and g == R - 1)
                    for h in range(2):
                        lhs = wtile[:, g * A + h * 128: g * A + h * 128 + 128]
                        for v in range(3):
                            nc.tensor.matmul(
                                uacc[v][h][:],
                                lhsT=lhs,
                                rhs=xts[v][:, g * D:(g + 1) * D],
                                start=first, stop=last)
                # d-trick: overwrite x1 <- x1-x3, x2 <- x2-x3 on the idle
                # DVE (phase 2 then needs one fewer op since sum(beta)==1).
                # The LAST batch's subs are deferred until after the PSUM
                # copies: emitted here they would sit on the DVE FIFO
                # between the final matmul and the collective input,
                # delaying the AllReduce by ~11 us; deferred, they run
                # inside the collective's idle window for free.
                if dtrick and b < NB - 1:
                    nc.vector.tensor_sub(xts[0][:], xts[0][:], xts[2][:])
                    nc.vector.tensor_sub(xts[1][:], xts[1][:], xts[2][:])
            for v in range(3):
                for h in range(2):
                    i = v * 2 + h
                    if h == 0:
                        nc.vector.tensor_copy(cc_in[:, i * D:(i + 1) * D],
                                              uacc[v][h][:])
                    else:
                        nc.scalar.copy(cc_in[:, i * D:(i + 1) * D],
                                       uacc[v][h][:])
            if dtrick:
                # deferred last-batch subs: overlap the collective
                bl = NB - 1
                nc.vector.tensor_sub(xt[(0, bl)][:], xt[(0, bl)][:],
                                     xt[(2, bl)][:])
                nc.vector.tensor_sub(xt[(1, bl)][:], xt[(1, bl)][:],
                                     xt[(2, bl)][:])

        # ---------------- all-reduce the GEMM partials ----------------------
        ccin_d = pdram.tile([128, 6 * D], FP16, name=f"ccin{r}",
                            tag=f"ccin{r}")
        if phase1_only:
            nc.sync.dma_start(ccin_d[:], cc_in[:])
            return
        ccout_d = pdram.tile([128, 6 * D], FP16, name=f"ccout{r}",
                             tag=f"ccout{r}")
        nc.sync.dma_start(ccin_d[:], cc_in[:])
        if collective:
            if ar_cols is not None:
                # timing probe: tiny-payload collective in the real context
                # (measures barrier latency; numerically wrong on purpose)
                ti = pdram.tile([128, ar_cols], FP16, name=f"ti{r}",
                                tag=f"ti{r}")
                to = pdram.tile([128, ar_cols], FP16, name=f"to{r}",
                                tag=f"to{r}")
                nc.sync.dma_start(ti[:], cc_in[:, 0:ar_cols])
                nc.gpsimd.collective_compute(
                    "AllReduce", mybir.AluOpType.add,
                    replica_groups=[list(range(n_cores))],
                    ins=[ti.opt()], outs=[to.opt()])
                nc.sync.dma_start(ccout_d[:], ccin_d[:])
                # overwrite cols 0:ar_cols with the AR result so the
                # downstream read depends on the collective
                nc.sync.dma_start(ccout_d[:, 0:ar_cols], to[:])
            else:
                nc.gpsimd.collective_compute(
                    "AllReduce", mybir.AluOpType.add,
                    replica_groups=[list(range(n_cores))],
                    ins=[ccin_d.opt()], outs=[ccout_d.opt()])
        else:
            nc.sync.dma_start(ccout_d[:], ccin_d[:])
        # reuse cc_in for the reduced result; tanh in place
        t_tanh = cc_in
        nc.sync.dma_start(t_tanh[:], ccout_d[:])

        # ---------------- tanh, scores, softmax, beta broadcast -------------
        nc.scalar.activation(t_tanh[:], t_tanh[:], Tanh)

        evs = []
        Bsb = []
        with (
            tc.tile_pool(name=f"ps_{r}", bufs=1, space="PSUM") as ps,
            tc.tile_pool(name=f"pB_{r}", bufs=1, space="PSUM") as pB,
        ):
            for v in range(3):
                s_ps = ps.tile([1, D], FP32, name=f"s{v}", tag=f"s{v}")
                for h in range(2):
                    i = v * 2 + h
                    nc.tensor.matmul(
                        s_ps[:], lhsT=hn16[:, h:h + 1],
                        rhs=t_tanh[:, i * D:(i + 1) * D],
                        start=(h == 0), stop=(h == 1))
                e_v = small.tile([1, D], FP32, name=f"e{v}", tag=f"e{v}")
                nc.scalar.activation(e_v[:], s_ps[:], Exp)
                evs.append(e_v)
            ssum = small.tile([1, D], FP32, name="ssum", tag="ssum")
            nc.vector.tensor_add(ssum[:], evs[0][:], evs[1][:])
            nc.vector.tensor_add(ssum[:], ssum[:], evs[2][:])
            rinv = small.tile([1, D], FP32, name="rinv", tag="rinv")
            nc.vector.reciprocal(rinv[:], ssum[:])
            # only beta0 and beta1 are needed: out = x3 + b0*d1 + b1*d2
            for v in range(2):
                b_v = small.tile([1, D], FP16, name=f"bt{v}", tag=f"bt{v}")
                nc.vector.tensor_mul(b_v[:], evs[v][:], rinv[:])
                B_ps = pB.tile([128, D], FP32, name=f"B{v}", tag=f"B{v}")
                nc.tensor.matmul(B_ps[:], lhsT=ones_sb[:], rhs=b_v[:],
                                 start=True, stop=True)
                B_v = small.tile([128, D], FP16, name=f"Bb{v}", tag=f"Bb{v}")
                nc.vector.tensor_copy(B_v[:], B_ps[:])
                Bsb.append(B_v)

        # ---------------- phase 2: out = x3 + b0*d1 + b1*d2 -----------------
        if not phase2:
            # timing-decomposition variant: emit a token store instead
            nc.sync.dma_start(outr[0][:, 0:6 * D],
                              t_tanh[:].bitcast(FP16)[0:125, 0:6 * D])
            return
        RH = R // 2
        if p2_full:
            # decomposition variant: full-batch-width ops (half the
            # instruction count), beta via stride-0 broadcast views
            Bb = [Bsb[v][0:P, :].unsqueeze(1).broadcast_to([P, R, D])
                  for v in range(2)]
            with (
                tc.tile_pool(name=f"pout_{r}", bufs=2) as pout,
                tc.tile_pool(name=f"ptmp_{r}", bufs=2) as ptmp,
            ):
                for b in range(NB):
                    d1 = xt[(0, b)][:].rearrange("p (r d) -> p r d", r=R)
                    d2 = xt[(1, b)][:].rearrange("p (r d) -> p r d", r=R)
                    x3 = xt[(2, b)][:].rearrange("p (r d) -> p r d", r=R)
                    ot = pout.tile([P, FW], FP16, name="o", tag="o")
                    tm = ptmp.tile([P, FW], FP16, name="t", tag="t")
                    o3 = ot[:].rearrange("p (r d) -> p r d", r=R)
                    t3 = tm[:].rearrange("p (r d) -> p r d", r=R)
                    e2 = nc.gpsimd if b < 4 else nc.vector
                    # muls need the stride-0 beta broadcast (3-D APs); the
                    # adds use flat contiguous 2-D APs so the DVE can pick
                    # its packed 2x mode
                    nc.vector.tensor_mul(o3, d1, Bb[0])
                    e2.tensor_mul(t3, d2, Bb[1])
                    nc.vector.tensor_add(ot[:], ot[:], xt[(2, b)][:])
                    nc.vector.tensor_add(ot[:], ot[:], tm[:])
                    eng = nc.sync if b % 2 == 0 else nc.scalar
                    if p2_tiny_out:
                        eng.dma_start(outr[b][:, 0:16], ot[:, 0:16])
                    else:
                        eng.dma_start(outr[b], ot[:])
            return
        # materialize beta repeated along r as flat contiguous [128, HW]
        # operands: plain 2D APs keep the DVE in its 2x packed mode (the
        # stride-0 broadcast middle dim risks dropping it to 1x)
        Brep = []
        for v in range(2):
            B_r = small.tile([128, HW], FP16, name=f"Br{v}", tag=f"Br{v}")
            src = Bsb[v][:].unsqueeze(1).broadcast_to([128, RH, D])
            nc.vector.tensor_copy(
                B_r[:].rearrange("p (r d) -> p r d", r=RH), src)
            Brep.append(B_r)
        # Pool is ~3.5x slower than DVE per TT op, so it gets only the
        # independent mul (op2, no chain) in 9 of 10 units: pool 9 ops,
        # DVE 31 ops -- both finish around the same time

        with (
            tc.tile_pool(name=f"pout_{r}", bufs=2) as pout,
            tc.tile_pool(name=f"ptmp_{r}", bufs=2) as ptmp,
        ):
            for b in range(NB):
                for u in range(2):
                    sl = slice(u * HW, (u + 1) * HW)
                    d1 = xt[(0, b)][:, sl]
                    d2 = xt[(1, b)][:, sl]
                    x3 = xt[(2, b)][:, sl]
                    ot = pout.tile([P, HW], FP16, name="o", tag="o")
                    tm = ptmp.tile([P, HW], FP16, name="t", tag="t")
                    ui = 2 * b + u
                    e2 = nc.gpsimd if (ui < 9 and not no_pool) else nc.vector
                    nc.vector.tensor_mul(ot[:], d1, Brep[0][0:P, :])
                    e2.tensor_mul(tm[:], d2, Brep[1][0:P, :])
                    nc.vector.tensor_add(ot[:], ot[:], x3)
                    nc.vector.tensor_add(ot[:], ot[:], tm[:])
                    eng = nc.sync if (2 * b + u) % 2 == 0 else nc.scalar
                    if p2_tiny_out:
                        eng.dma_start(outr[b][:, u * 16:(u + 1) * 16],
                                      ot[:, 0:16])
                    else:
                        eng.dma_start(outr[b][:, sl], ot[:])


def build_bass(n_cores=N_CORES, collective=True, repeat=1, phase2=True,
               dtrick=True, phase1_only=False, ar_cols=None, no_pool=False,
               p2_full=True, p2_tiny_out=False):
    nc = bacc.Bacc("TRN2", target_bir_lowering=False, debug=False,
                   num_devices=n_cores)

    x1 = nc.dram_tensor("x1", [N_LOC, D], FP16, kind="ExternalInput")
    x2 = nc.dram_tensor("x2", [N_LOC, D], FP16, kind="ExternalInput")
    x3 = nc.dram_tensor("x3", [N_LOC, D], FP16, kind="ExternalInput")
    wt = nc.dram_tensor("wt", [N_LOC, A], FP16, kind="ExternalInput")
    hnt = nc.dram_tensor("hnt", [A, 1], FP32, kind="ExternalInput")
    out = nc.dram_tensor("out", [N_LOC, D], FP16, kind="ExternalOutput")

    with tile.TileContext(nc) as tc:
        with (
            tc.tile_pool(name="smallg", bufs=1) as smallg,
            tc.tile_pool(name="pdram", bufs=1, space="DRAM") as pdram,
        ):
            x1r = x1.ap().rearrange("(b p r) d -> b p (r d)", p=P, r=R)
            x2r = x2.ap().rearrange("(b p r) d -> b p (r d)", p=P, r=R)
            x3r = x3.ap().rearrange("(b p r) d -> b p (r d)", p=P, r=R)
            wtr = wt.ap().rearrange("(b p r) a -> b p (r a)", p=P, r=R)
            outr = out.ap().rearrange("(b p r) d -> b p (r d)", p=P, r=R)
            xrs = (x1r, x2r, x3r)

            # h_n laid out [a_half(128 partitions), h(2)]
            hn_sb = smallg.tile([128, 2], FP32, tag="hn")
            nc.sync.dma_start(hn_sb[:, :],
                              hnt.ap().rearrange("(h a) o -> a (h o)", h=2))
            hn16 = smallg.tile([128, 2], FP16, tag="hn16")
            nc.vector.tensor_copy(hn16[:], hn_sb[:])
            ones_sb = smallg.tile([1, 128], FP16, tag="ones")
            nc.vector.memset(ones_sb[:], 1.0)

            for rep in range(repeat):
                _emit_iteration(nc, tc, rep, xrs, wtr, outr, hn16, ones_sb,
                                pdram, n_cores, collective, phase2, dtrick,
                                phase1_only, ar_cols, no_pool,
                                p2_full, p2_tiny_out)

    nc.compile()
    return nc


_NC_CACHE = {}


def _get_nc():
    if "nc" not in _NC_CACHE:
        _NC_CACHE["nc"] = build_bass()
    return _NC_CACHE["nc"]


def kernel(x1, x2, x3, W, h_n):
    x1h = np.ascontiguousarray(x1, dtype=np.float16)
    x2h = np.ascontiguousarray(x2, dtype=np.float16)
    x3h = np.ascontiguousarray(x3, dtype=np.float16)
    Wh = np.ascontiguousarray(W, dtype=np.float16)
    h_n = np.ascontiguousarray(h_n, dtype=np.float32)

    hnt = np.ascontiguousarray(h_n.reshape(-1)[:, None])  # (A, 1)
    in_maps = []
    for c in range(N_CORES):
        sl = slice(c * N_LOC, (c + 1) * N_LOC)
        in_maps.append({
            "x1": x1h[sl],
            "x2": x2h[sl],
            "x3": x3h[sl],
            "wt": np.ascontiguousarray(Wh[:, sl].T),
            "hnt": hnt,
        })

    nc = _get_nc()
    res = run_bass_kernel_spmd(nc, in_maps, core_ids=list(range(N_CORES)))
    out = np.concatenate([res.results[c]["out"] for c in range(N_CORES)],
                         axis=0)
    return out.astype(np.float32)
